# revision 93
# baseline (speedup 1.0000x reference)
"""Phi3 decoder layer on 8 Trainium2 NeuronCores (tensor-parallel).

Sharding: qkv/gate_up column-sharded, o/down row-sharded over 8 cores
(4 q-heads + 1 kv-head per core). v3 restructure vs v2 baseline:
  - x / hs activations stored p-major ([128, HC*ST]) in DRAM so every
    tile load/store is line-contiguous (descriptor-gen was starving
    the weight DMAs through the sync queue in v2)
  - per pair: attn(s0), attn(s1), THEN oproj(s0), oproj(s1) so the
    softmax epilogue (ACT/DVE) of tile s is covered by attention MMs
    of tile s+1 instead of stalling the o-proj matmuls
  - attention emits the PV matmul two k-blocks behind the QK matmul
    so the probs exp (ACT) latency never bubbles the PE
  - o-proj residual add moved from the PE (ident matmul) to a DVE
    scalar_tensor_tensor during psum evacuation (-96 matmuls)
  - weight streams split across the sync AND scalar HW-DGE queues
    (wg||wu, wd/wom alternating) with deeper prefetch pools
  - final tile's down-proj split in row halves with two ReduceScatters
    so only half a RS is tail-exposed; host reassembles the permuted
    shard rows
"""
import math

import numpy as np
import ml_dtypes

import concourse.bass as bass
import concourse.tile as tile
import concourse.mybir as mybir
from concourse import bass_utils
from concourse.tile import ScopedClock

# ---------------------------------------------------------------- constants
B, S, HID = 1, 2048, 3072
NH, NKV, D = 32, 8, 96
INTER = 8192
EPS = 1e-5
NCORES = 8
QH = NH // NCORES            # 4 q heads per core
DMC = QH * D                 # 384 attn model dims per core (3 x 128)
DIC = INTER // NCORES        # 1024 down rows per core (8 x 128)
HC = HID // 128              # 24 hid chunks
ST = 512                     # s tile
NST = S // ST                # 4
KC = 128                     # k chunk in attention
NKC = S // KC                # 16
SM_SCALE = 1.0 / math.sqrt(D)
NEG = -1e30
HHC = HC // 2                # 12 hid chunks per weight half

F32 = mybir.dt.float32
BF16 = mybir.dt.bfloat16
AF = mybir.ActivationFunctionType
ALU = mybir.AluOpType

# ------------------------------------------------------- walrus workarounds
# This walrus build encodes at most ONE sync wait per instruction. Tile's
# exit drain and any multi-producer instruction exceed that; split extra
# waits onto single-wait NoOps on the same (in-order) engine.
_split_counter = [0]


def _patched_drain_and_barrier(self, tick_clock, wait_clock):
    drain_inst = self.nc.sync.drain()
    wait_clock.add_sem_waits(
        drain_inst.ins, ScopedClock({None: tick_clock.global_clock})
    )
    si = drain_inst.ins.sync_info
    if si is not None and si.on_wait and len(si.on_wait) > 1:
        waits = list(si.on_wait)
        upd = list(si.on_update) if si.on_update else []
        drain_inst.ins.sync_info = mybir.SyncInfo(on_wait=[waits[0]], on_update=upd)
        for w in waits[1:]:
            n = self.nc.sync.nop()
            n.ins.sync_info = mybir.SyncInfo(on_wait=[w], on_update=[])
    self.nc.all_engine_barrier()
    assert self.sems is not None
    popped = self.nc._tile_sem_poison_stack.pop()
    assert popped is self._sem_poison
    self.nc.clear_and_free_semaphores(list(self.sems.allocated().values()))
    self.nc.all_engine_barrier()


def _split_multi_waits(nc):
    for fn in nc.m.functions:
        for bb in fn.blocks:
            insts = list(bb.instructions)
            out = []
            changed = False
            for inst in insts:
                si = inst.sync_info
                if si is not None and si.on_wait and len(si.on_wait) > 1:
                    waits = list(si.on_wait)
                    upd = list(si.on_update) if si.on_update else []
                    for w in waits[:-1]:
                        _split_counter[0] += 1
                        n = mybir.InstNoOp(
                            name=f"I-waitsplit-{_split_counter[0]}", ins=[], outs=[]
                        )
                        n.engine = inst.engine
                        n.sync_info = mybir.SyncInfo(on_wait=[w], on_update=[])
                        out.append(n)
                    inst.sync_info = mybir.SyncInfo(on_wait=[waits[-1]], on_update=upd)
                    changed = True
                out.append(inst)
            if changed:
                bb.instructions = out


tile.TileContext._drain_and_barrier = _patched_drain_and_barrier

# ------------------------------------------------------------- kernel build

PAIRS = ((0, 1), (2, 3))
RS_SPLIT_ROWS = HID // 2     # st3 down-proj RS split point (rows)


def build_nc(attn_table, nbias):
    """attn_table[st] = list of (kchunk, bias_idx) with bias_idx=-1 for fully
    open blocks; nbias = number of bias patterns (>=1)."""
    nc = bass.Bass("TRN2", num_devices=NCORES)

    # x pre-packed p-major on host: xP[st, p, c*ST+s] = x[c*128+p, st*ST+s]
    xP = nc.dram_tensor("xP", [NST, 128, HC * ST], BF16, kind="ExternalInput")
    wqkv = nc.dram_tensor("wqkv", [QH + 2, 128, HC * D], BF16, kind="ExternalInput")
    wo = nc.dram_tensor("wo", [HC, 128, 3 * 128], BF16, kind="ExternalInput")
    wgu_g = nc.dram_tensor("wgu_g", [DIC // 128, 128, HID], BF16, kind="ExternalInput")
    wgu_u = nc.dram_tensor("wgu_u", [DIC // 128, 128, HID], BF16, kind="ExternalInput")
    wd = nc.dram_tensor("wd", [HC, 128, DIC], BF16, kind="ExternalInput")
    sinT = nc.dram_tensor("sinT", [D, S], BF16, kind="ExternalInput")
    cosT = nc.dram_tensor("cosT", [D, S], BF16, kind="ExternalInput")
    ident_in = nc.dram_tensor("ident", [128, 128], BF16, kind="ExternalInput")
    pmat_in = nc.dram_tensor("pmat", [D, D], BF16, kind="ExternalInput")
    biasp = nc.dram_tensor("biasp", [128, nbias, ST], BF16, kind="ExternalInput")
    out_shard = nc.dram_tensor("out_shard", [DMC, 2 * ST], BF16,
                               kind="ExternalOutput")
    ohalf = {}
    for st in (2, 3):
        for h in ("a", "b"):
            ohalf[(st, h)] = nc.dram_tensor(
                f"o{st}{h}", [RS_SPLIT_ROWS // NCORES, ST], BF16,
                kind="ExternalOutput")

    # o_in / hs_sh p-major: element (p, c, s) at [p, c*ST+s]
    o_in = [nc.dram_tensor(f"o_in{st}", [128, HC * ST], BF16)
            for st in range(NST)]
    hs_sh = [
        nc.dram_tensor(f"hs_sh{st}", [128, HC * ST], BF16, addr_space="Shared")
        for st in range(NST)
    ]
    d_in = [nc.dram_tensor(f"d_in{st}", [HID, ST], BF16) for st in range(NST)]
    rs_o = [nc.dram_tensor(f"rs_o{st}", [DMC, ST], BF16) for st in range(2)]
    rs_half = {}
    for st in (2, 3):
        for h in ("a", "b"):
            rs_half[(st, h)] = nc.dram_tensor(
                f"rs_{st}{h}", [RS_SPLIT_ROWS // NCORES, ST], BF16)
    rg = [list(range(NCORES))]

    with tile.TileContext(nc) as tc:
        with (
            tc.tile_pool(name="const", bufs=1) as consts,
            tc.tile_pool(name="xh", bufs=1) as xh,
            tc.tile_pool(name="qt", bufs=1) as qtp,
            tc.tile_pool(name="actp", bufs=1) as actp,
            tc.tile_pool(name="wpool", bufs=1) as wpool,
            tc.tile_pool(name="work", bufs=2) as work,
            tc.tile_pool(name="psA", bufs=2, space="PSUM") as psA,
            tc.tile_pool(name="psB", bufs=3, space="PSUM") as psB,
            tc.tile_pool(name="psC", bufs=2, space="PSUM") as psC,
            tc.tile_pool(name="psD", bufs=1, space="PSUM") as psD,
        ):
            # ---------------- persistent constants
            sin_sb = consts.tile([D, S], BF16, name="sin_sb")
            cos_sb = consts.tile([D, S], BF16, name="cos_sb")
            ident = consts.tile([128, 128], BF16, name="ident")
            pmat = consts.tile([D, D], BF16, name="pmat")
            bias_sb = consts.tile([128, nbias, ST], BF16, name="bias_sb")

            onesb = consts.tile([128, 1], BF16, name="onesb")
            nc.vector.memset(onesb[:], 1.0)
            ones1 = consts.tile([1, 128], BF16, name="ones1")
            nc.vector.memset(ones1[:], 1.0)
            epsc = consts.tile([1, 1], F32, name="epsc")
            nc.vector.memset(epsc[:], EPS)
            KT = consts.tile([D, S], BF16, name="KT")
            Vk = consts.tile([128, NKC, D + 1], BF16, name="Vk")
            nc.vector.memset(Vk[:, :, D:D + 1], 1.0)

            def load_x(st, engines, nch=2):
                """x tile st from packed xP: nch contiguous chunk DMAs
                round-robined over engines. Fine chunks at startup let the
                first qkv matmuls begin before the whole tile lands."""
                t = xh.tile([128, HC, ST], BF16, name=f"x{st}", tag="xh",
                            bufs=4)
                cw = HC // nch
                for g in range(nch):
                    eng = engines[g % len(engines)]
                    eng.dma_start(
                        t[:, g * cw:(g + 1) * cw, :],
                        xP.ap()[st, :, g * cw * ST:(g + 1) * cw * ST],
                    )
                return t

            def load_h(st):
                """hs tile from p-major shared buffer; gpsimd so the
                AllReduce-gated wait can't block sync/scalar queues."""
                t = xh.tile([128, HC, ST], BF16, name=f"h{st}", tag="xh",
                            bufs=4)
                for g in range(2):
                    nc.gpsimd.dma_start(
                        t[:, g * HHC:(g + 1) * HHC, :],
                        hs_sh[st].ap()[:, g * HHC * ST:(g + 1) * HHC * ST],
                    )
                return t

            def load_consts():
                nc.sync.dma_start(pmat[:], pmat_in.ap())
                nc.sync.dma_start(ident[:], ident_in.ap())
                nc.scalar.dma_start(sin_sb[:], sinT.ap())
                nc.scalar.dma_start(cos_sb[:], cosT.ap())
                nc.gpsimd.dma_start(bias_sb[:], biasp.ap())

            def stats(t, tag):
                """rstd broadcast tile [128, ST] bf16 from raw tile t.
                Squares + chunk reduction on DVE; rsqrt as exp(-0.5*ln(var))
                so ACT stays on the exp table set (no sqrt-set reloads)."""
                acc = work.tile([128, ST], BF16, name="acc", tag="acc", bufs=1)
                for hcx in range(HC):
                    # squares on ACT (idle in these windows), adds on DVE:
                    # halves the DVE cost of each rmsnorm stats pass
                    xsq = work.tile([128, ST], BF16, name="xsq", tag="xsq", bufs=2)
                    nc.scalar.activation(xsq[:], t[:, hcx, :], AF.Square)
                    if hcx == 0:
                        nc.vector.tensor_copy(acc[:], xsq[:])
                    else:
                        nc.vector.tensor_add(acc[:], acc[:], xsq[:])
                pss = psD.tile([1, ST], F32, name="pss", tag="psD")
                nc.tensor.matmul(pss[:], onesb[:], acc[:], start=True, stop=True)
                lvar = work.tile([1, ST], F32, name="lvar", tag="sc1", bufs=1)
                nc.scalar.activation(lvar[:], pss[:], AF.Ln,
                                     scale=1.0 / HID, bias=epsc[0:1, 0:1])
                rstdb = work.tile([1, ST], BF16, name="rstdb", tag="sc1b", bufs=1)
                nc.scalar.activation(rstdb[:], lvar[:], AF.Exp, scale=-0.5)
                pbc = psD.tile([128, ST], F32, name="pbc", tag="psD")
                nc.tensor.matmul(pbc[:], ones1[:], rstdb[:], start=True, stop=True)
                bc = work.tile([128, ST], BF16, name=tag, tag="rbc", bufs=4)
                nc.scalar.copy(bc[:], pbc[:])
                return bc

            def do_rope(qs, dst, st):
                """dst [D, ST] bf16 <- rope(qs [D, ST] bf16 sbuf) at s-tile st.
                rotate_half is a signed 96x96 permutation done on the PE."""
                sl = slice(st * ST, (st + 1) * ST)
                prot = psD.tile([D, ST], F32, name="prot", tag="psD")
                nc.tensor.matmul(prot[:], pmat[:], qs[:], start=True, stop=True)
                tcs = work.tile([D, ST], BF16, name="tcs", tag="rope2")
                nc.vector.tensor_mul(tcs[:], qs[:], cos_sb[:, sl])
                trs = work.tile([D, ST], BF16, name="trs", tag="rope2")
                nc.vector.tensor_mul(trs[:], prot[:], sin_sb[:, sl])
                nc.vector.tensor_add(dst, tcs[:], trs[:])

            def do_vtr(vt, st):
                for c4 in range(ST // 128):
                    ptr = psD.tile([128, D], BF16, name="ptr", tag="psD")
                    nc.tensor.transpose(
                        ptr[:], vt[:, c4 * 128:(c4 + 1) * 128], ident[0:D, 0:D]
                    )
                    nc.vector.tensor_copy(Vk[:, st * 4 + c4, 0:D], ptr[:])

            def load_wq(m):
                wq = wpool.tile([128, HC * D], BF16, name="wq", tag="wq",
                                bufs=2)
                eng = nc.scalar if m % 2 else nc.sync
                eng.dma_start(wq[:], wqkv.ap()[m])
                return wq

            def _flush_one(pend):
                if pend:
                    kind, a, b, c = pend.pop(0)
                    if kind == "rope":
                        do_rope(a, b, c)
                    else:
                        do_vtr(a, c)

            def qkv_part(G, xts, r1s, QTs, mlist, pend, wq0=None,
                         hooks=None):
                """Weight-stationary qkv + rope over the 2 tiles of pair G
                for the given m iterations; pend carries the lazy rope/vtr
                queue across parts so the stream can be split around other
                PE work (o-proj) to fill its evacuation-convoy stalls."""

                def flush_one():
                    _flush_one(pend)

                for m in mlist:
                    wq = wq0 if (m == 0 and wq0 is not None) else load_wq(m)
                    for st in G:
                        pq = psA.tile([D, ST], F32, name="pq", tag="psA")
                        for hcx in range(HC):
                            nc.tensor.matmul(
                                pq[:], wq[:, hcx * D:(hcx + 1) * D],
                                xts[st][:, hcx, :],
                                start=(hcx == 0), stop=(hcx == HC - 1),
                            )
                        if m < QH:
                            qs = work.tile([D, ST], BF16, name="qs", tag="qs",
                                           bufs=2)
                            nc.vector.tensor_mul(qs[:], pq[:], r1s[st][0:D, :])
                            flush_one()
                            pend.append(("rope", qs, QTs[st][:, m, :], st))
                        elif m == QH:
                            qs = work.tile([D, ST], BF16, name="qs", tag="qs",
                                           bufs=2)
                            nc.vector.tensor_mul(qs[:], pq[:], r1s[st][0:D, :])
                            flush_one()
                            pend.append(
                                ("rope", qs, KT[:, st * ST:(st + 1) * ST], st))
                        else:
                            vt = work.tile([D, ST], BF16, name="vt", tag="qs",
                                           bufs=2)
                            nc.vector.tensor_mul(vt[:], pq[:], r1s[st][0:D, :])
                            flush_one()
                            pend.append(("vtr", vt, None, st))
                    if hooks and m in hooks:
                        hooks[m]()

            def qkv_pair(G, xts, r1s, QTs, wq0=None, hooks=None):
                pend = []
                qkv_part(G, xts, r1s, QTs, range(QH + 2), pend, wq0=wq0,
                         hooks=hooks)
                while pend:
                    _flush_one(pend)

            def finish_head(pa, h, a3):
                ldn = work.tile([1, ST], F32, name="ldn", tag="sc1", bufs=1)
                nc.scalar.activation(ldn[:], pa[D:D + 1, :], AF.Ln)
                recb = work.tile([1, ST], BF16, name="recb", tag="sc1b", bufs=1)
                nc.scalar.activation(recb[:], ldn[:], AF.Exp, scale=-1.0)
                pbc2 = psD.tile([D, ST], F32, name="pbc2", tag="psD")
                nc.tensor.matmul(pbc2[:], ones1[:, 0:D], recb[:],
                                 start=True, stop=True)
                bcs = work.tile([D, ST], BF16, name="bcs", tag="bcs", bufs=1)
                nc.vector.tensor_copy(bcs[:], pbc2[:])
                # scatter h-th head rows (96h..96h+96) into 128-row tiles
                r0 = h * D
                r1 = r0 + D
                j0, j1 = r0 // 128, (r1 - 1) // 128
                for j in range(j0, j1 + 1):
                    lo = max(r0, j * 128)
                    hi = min(r1, (j + 1) * 128)
                    # partition-offset accesses may span at most 32
                    # partitions unless they start at 0 -> 32-row pieces
                    for p0 in range(lo, hi, 32):
                        p1 = min(p0 + 32, hi)
                        nc.vector.tensor_mul(
                            a3[j][p0 - j * 128:p1 - j * 128, :],
                            pa[p0 - r0:p1 - r0, :],
                            bcs[p0 - r0:p1 - r0, :],
                        )

            def attn(st, QT, tag):
                """Flash attention for tile st. The PV matmul for k-block i
                is emitted while the QK matmul for block i+2 runs, so the
                probs exp on ACT never stalls the PE."""
                a3 = [
                    work.tile([128, ST], BF16, name=f"a3_{j}{tag}",
                              tag=f"a3_{j}", bufs=2)
                    for j in range(3)
                ]
                # biased (diagonal) blocks first: their extra DVE mul stage
                # then has the rest of the k-stream as latency cover
                blocks = sorted(attn_table[st], key=lambda b: b[1] < 0)
                nb = len(blocks)
                pend_head = None
                for h in range(QH):
                    pa = psC.tile([D + 1, ST], F32, name="pa", tag="psC")
                    pv = []          # pending (kc, probs, bi)

                    def flush_pv(pv=pv, pa=pa):
                        kc, probs, bi = pv.pop(0)
                        nc.tensor.matmul(
                            pa[:], Vk[:, kc, :], probs[:],
                            start=(bi == 0), stop=(bi == nb - 1),
                        )

                    for bi, (kc, bidx) in enumerate(blocks):
                        ps = psB.tile([128, ST], F32, name="ps", tag="psB")
                        nc.tensor.matmul(
                            ps[:], KT[:, kc * KC:(kc + 1) * KC],
                            QT[:, h, :], start=True, stop=True,
                        )
                        probs = work.tile([128, ST], BF16, name="probs",
                                          tag="probs", bufs=3)
                        nc.scalar.activation(probs[:], ps[:], AF.Exp,
                                             scale=SM_SCALE)
                        if bidx >= 0:
                            nc.vector.tensor_mul(probs[:], probs[:],
                                                 bias_sb[:, bidx, :])
                        pv.append((kc, probs, bi))
                        if len(pv) > 2:
                            flush_pv()
                    while pv:
                        flush_pv()
                    if pend_head is not None:
                        finish_head(pend_head, h - 1, a3)
                    pend_head = pa
                finish_head(pend_head, QH - 1, a3)
                return a3

            def oproj(st, xt, a3):
                """o-proj + residual; wom weights stream on both HW-DGE
                queues with deep prefetch; residual x/8 added on DVE.
                Evacuations land in a 4-wide staging tile flushed as one
                contiguous DMA to cut queue descriptor pressure."""
                ob4 = None
                for m in range(HC):
                    wom = wpool.tile([128, 3 * 128], BF16, name="wom", tag="wo",
                                     bufs=4)
                    weng = nc.sync if m % 2 else nc.scalar
                    weng.dma_start(wom[:], wo.ap()[m])
                    # psB is idle during o-proj: its 3-deep rotation absorbs
                    # DVE evacuation jitter, and decouples this stream from
                    # the interleaved qkv pq rotation on psA
                    po = psB.tile([128, ST], F32, name="po", tag="psB")
                    for j in range(3):
                        nc.tensor.matmul(
                            po[:], wom[:, j * 128:(j + 1) * 128], a3[j][:],
                            start=(j == 0), stop=(j == 2),
                        )
                    if m % 4 == 0:
                        ob4 = work.tile([128, 4, ST], BF16, name="ob4",
                                        tag="ob4", bufs=2)
                    ob = ob4[:, m % 4, :]
                    # evacuation + residual x/8: fused DVE STT on even m;
                    # ACT copy + cheap SBUF-only DVE add on odd m
                    if m % 2 == 0:
                        nc.vector.scalar_tensor_tensor(
                            ob, xt[:, m, :], 1.0 / NCORES, po[:],
                            op0=ALU.mult, op1=ALU.add,
                        )
                    else:
                        nc.scalar.copy(ob, po[:])
                        nc.vector.scalar_tensor_tensor(
                            ob, xt[:, m, :], 1.0 / NCORES, ob,
                            op0=ALU.mult, op1=ALU.add,
                        )
                    if m % 4 == 3:
                        oeng = nc.scalar if (m // 4) % 2 else nc.sync
                        oeng.dma_start(
                            o_in[st].ap()[:, (m - 3) * ST:(m + 1) * ST],
                            ob4[:],
                        )
                nc.gpsimd.collective_compute(
                    "AllReduce", ALU.add, replica_groups=rg,
                    ins=[o_in[st].ap().opt()], outs=[hs_sh[st].ap().opt()],
                )

            def gateup_pair(G, hts, r2s, acts, pre=None):
                """gate/up projections; wg streams on sync, wu on scalar,
                both in HID/2 halves for finer prefetch + less SBUF.
                pre = preloaded (wgh, wuh) for gm=0."""
                for gm in range(DIC // 128):
                    if gm == 0 and pre is not None:
                        wgh, wuh = pre
                    else:
                        wgh, wuh = load_gu(gm)
                    for st in G:
                        pg = psA.tile([128, ST], F32, name="pg", tag="psA")
                        pu = psC.tile([128, ST], F32, name="pu", tag="psC")
                        for hcx in range(HC):
                            nc.tensor.matmul(
                                pg[:], wgh[hcx // HHC][
                                    :, (hcx % HHC) * 128:(hcx % HHC + 1) * 128],
                                hts[st][:, hcx, :],
                                start=(hcx == 0), stop=(hcx == HC - 1),
                            )
                        for hcx in range(HC):
                            nc.tensor.matmul(
                                pu[:], wuh[hcx // HHC][
                                    :, (hcx % HHC) * 128:(hcx % HHC + 1) * 128],
                                hts[st][:, hcx, :],
                                start=(hcx == 0), stop=(hcx == HC - 1),
                            )
                        gr = work.tile([128, ST], BF16, name="gr", tag="gu2",
                                       bufs=2)
                        nc.vector.tensor_mul(gr[:], pg[:], r2s[st][:])
                        nc.scalar.activation(gr[:], gr[:], AF.Silu)
                        ur = work.tile([128, ST], BF16, name="ur", tag="gu2",
                                       bufs=2)
                        nc.vector.tensor_mul(ur[:], pu[:], r2s[st][:])
                        nc.vector.tensor_mul(acts[st][:, gm, :], gr[:], ur[:])

            db4s = {}

            def down_m(m, sts, hts, acts):
                """One wd row-block of the down-proj for the given tiles.
                d_in writes are staged 4 m's at a time into one contiguous
                512-row DMA."""
                wdm = wpool.tile([128, DIC], BF16, name="wdm", tag="wd",
                                 bufs=2)
                weng = nc.sync if m % 2 else nc.scalar
                weng.dma_start(wdm[:], wd.ap()[m])
                for st in sts:
                    # psB/psC are idle during down: each tile of the pair
                    # gets its own accumulator rotation, riding out d_in/
                    # ReduceScatter DMA bursts delaying the DVE STTs
                    if st == sts[0]:
                        pd = psB.tile([128, ST], F32, name="pd", tag="psB")
                    else:
                        pd = psC.tile([128, ST], F32, name="pd", tag="psC")
                    for ic in range(DIC // 128):
                        nc.tensor.matmul(
                            pd[:], wdm[:, ic * 128:(ic + 1) * 128],
                            acts[st][:, ic, :],
                            start=(ic == 0), stop=(ic == DIC // 128 - 1),
                        )
                    if m % 4 == 0:
                        db4s[st] = work.tile([128, 4, ST], BF16, name="db4",
                                             tag="ob4", bufs=2)
                    nc.vector.scalar_tensor_tensor(
                        db4s[st][:, m % 4, :], hts[st][:, m, :], 1.0 / NCORES,
                        pd[:], op0=ALU.mult, op1=ALU.add,
                    )
                    if m % 4 == 3:
                        oeng = nc.scalar if (m // 4 + st) % 2 else nc.sync
                        oeng.dma_start(
                            d_in[st].ap()[(m - 3) * 128:(m + 1) * 128, :]
                            .rearrange("(c p) s -> p c s", p=128),
                            db4s[st][:],
                        )

            def rs_tile(st):
                ssl = slice(st * ST, (st + 1) * ST)
                nc.gpsimd.collective_compute(
                    "ReduceScatter", ALU.add, replica_groups=rg,
                    ins=[d_in[st].ap().opt()], outs=[rs_o[st].ap().opt()],
                )
                nc.sync.dma_start(out_shard.ap()[:, ssl], rs_o[st].ap())

            def rs_half_tile(st, half):
                r0 = 0 if half == "a" else RS_SPLIT_ROWS
                nc.gpsimd.collective_compute(
                    "ReduceScatter", ALU.add, replica_groups=rg,
                    ins=[d_in[st].ap()[r0:r0 + RS_SPLIT_ROWS, :].opt()],
                    outs=[rs_half[(st, half)].ap().opt()],
                )
                eng = nc.sync if half == "a" else nc.scalar
                eng.dma_start(ohalf[(st, half)].ap(), rs_half[(st, half)].ap())

            def load_gu(gm):
                """Prefetchable gate/up weight halves for one gm block."""
                wgh, wuh = [], []
                for g in range(2):
                    w1 = wpool.tile([128, HHC * 128], BF16, name="wg",
                                    tag="wg", bufs=3)
                    nc.sync.dma_start(
                        w1[:], wgu_g.ap()[gm, :, g * HHC * 128:
                                          (g + 1) * HHC * 128])
                    wgh.append(w1)
                    w2 = wpool.tile([128, HHC * 128], BF16, name="wu",
                                    tag="wu", bufs=3)
                    nc.scalar.dma_start(
                        w2[:], wgu_u.ap()[gm, :, g * HHC * 128:
                                          (g + 1) * HHC * 128])
                    wuh.append(w2)
                return wgh, wuh

            # ================= main program =================
            # sync: wq0, x halves, pmat/ident | scalar: x halves, sin/cos |
            # gpsimd: bias, x2, x3. All contiguous single-line DMAs.
            wq0 = load_wq(0)
            xts, r1s, QTs = {}, {}, {}
            xts[0] = load_x(0, (nc.sync, nc.scalar, nc.gpsimd), nch=8)
            xts[1] = load_x(1, (nc.sync, nc.scalar, nc.gpsimd), nch=8)
            load_consts()
            xts[2] = load_x(2, (nc.gpsimd,))
            xts[3] = load_x(3, (nc.gpsimd,))
            # ---- pair G0: qkv, attn, oproj ----
            r1s[0] = stats(xts[0], "r1")
            r1s[1] = stats(xts[1], "r1")
            for st in (0, 1):
                QTs[st] = qtp.tile([D, QH, ST], BF16, name=f"QT{st}",
                                   tag="QT", bufs=2)
            qkv_pair((0, 1), xts, r1s, QTs, wq0=wq0,
                     hooks={4: lambda: r1s.__setitem__(
                         2, stats(xts[2], "r1"))})
            r1s[3] = stats(xts[3], "r1")
            a3s = {}
            a3s[0] = attn(0, QTs[0], tag="0")
            a3s[1] = attn(1, QTs[1], tag="1")
            wq0_next = load_wq(0)
            for st in (2, 3):
                QTs[st] = qtp.tile([D, QH, ST], BF16, name=f"QT{st}",
                                   tag="QT", bufs=2)
            # interleave the independent qkv-G1 matmul stream into the
            # o-proj windows: the PE grinds qkv MMs while DVE drains the
            # o-proj evacuation backlog instead of stalling on it
            pend1 = []
            oproj(0, xts[0], a3s[0])
            qkv_part((2, 3), xts, r1s, QTs, (0, 1, 2), pend1, wq0=wq0_next)
            oproj(1, xts[1], a3s[1])
            qkv_part((2, 3), xts, r1s, QTs, (3, 4, 5), pend1)
            while pend1:
                _flush_one(pend1)
            # ---- pair G1: attn, oproj ----
            a3s[2] = attn(2, QTs[2], tag="0")
            a3s[3] = attn(3, QTs[3], tag="1")
            # prefetch the first MLP gate/up block ahead of the last
            # o-proj writes + AR traffic
            pre_gu = load_gu(0)
            oproj(2, xts[2], a3s[2])
            oproj(3, xts[3], a3s[3])

            # MLP in pairs; hs loads issued on gpsimd right after the AR
            # they depend on so they run during the preceding compute.
            hts, r2s, acts = {}, {}, {}
            for Gi, G in enumerate(PAIRS):
                for st in G:
                    hts[st] = load_h(st)
                for st in G:
                    r2s[st] = stats(hts[st], "r2")
                for st in G:
                    acts[st] = actp.tile([128, DIC // 128, ST], BF16,
                                         name=f"act{st}", tag="act", bufs=2)
                gateup_pair(G, hts, r2s, acts, pre=pre_gu)
                pre_gu = None
                if Gi == 0:
                    # prefetch G1's first gate/up block ahead of the down
                    # d_in/wd traffic, then merged-pair down; RS0/RS1 hide
                    # behind the second MLP pair's compute
                    pre_gu = load_gu(0)
                    for m in range(HC):
                        down_m(m, G, hts, acts)
                    rs_tile(G[0])
                    rs_tile(G[1])
                else:
                    # merged pair (one wd pass for both tiles); the first
                    # row-halves of both tiles finish at m=11 so their RS
                    # hides behind the second half, leaving only two
                    # half-RS ops tail-exposed
                    for m in range(HC):
                        down_m(m, G, hts, acts)
                        if m == HC // 2 - 1:
                            rs_half_tile(2, "a")
                            rs_half_tile(3, "a")
                    rs_half_tile(2, "b")
                    rs_half_tile(3, "b")

    _split_multi_waits(nc)
    return nc


# --------------------------------------------------------------- host side
_NC_CACHE = {}


def _get_nc(table_key, attn_table, nbias):
    if table_key not in _NC_CACHE:
        _NC_CACHE[table_key] = build_nc(attn_table, nbias)
    return _NC_CACHE[table_key]


def kernel(hidden_states, sin, cos, attention_mask, position_ids,
           qkv_kernel, o_kernel, gate_up_kernel, down_kernel, ln1_w, ln2_w):
    hidden_states = np.asarray(hidden_states)
    sin = np.asarray(sin)
    cos = np.asarray(cos)
    attention_mask = np.asarray(attention_mask)
    position_ids = np.asarray(position_ids)
    qkv_kernel = np.asarray(qkv_kernel, np.float32)
    o_kernel = np.asarray(o_kernel, np.float32)
    gate_up_kernel = np.asarray(gate_up_kernel, np.float32)
    down_kernel = np.asarray(down_kernel, np.float32)
    ln1_w = np.asarray(ln1_w, np.float32)
    ln2_w = np.asarray(ln2_w, np.float32)

    bf = ml_dtypes.bfloat16
    # mask -> per-block classification (q-tile 512 x k-chunk 128)
    mask = np.asarray(attention_mask[0, 0])  # [S(q), S(k)]
    patterns = {}
    pat_arrays = []
    attn_table = []
    for st in range(NST):
        rows = []
        sub_q = mask[st * ST:(st + 1) * ST, :]
        for kc in range(NKC):
            blk = sub_q[:, kc * KC:(kc + 1) * KC]  # [512 q, 128 k]
            if blk.min() > 0:
                rows.append((kc, -1))
            elif blk.max() <= 0:
                continue
            else:
                bt = np.where(blk.T > 0, np.float32(1.0),
                              np.float32(0.0)).astype(bf)  # [128 k, 512 q]
                key = bt.tobytes()
                if key not in patterns:
                    patterns[key] = len(pat_arrays)
                    pat_arrays.append(bt)
                rows.append((kc, patterns[key]))
        attn_table.append(tuple(rows))
    nbias = max(1, len(pat_arrays))
    if not pat_arrays:
        pat_arrays = [np.zeros((KC, ST), bf)]
    biasp = np.stack(pat_arrays, axis=1)  # [128, nbias, 512]

    table_key = (tuple(attn_table), nbias)
    nc = _get_nc(table_key, attn_table, nbias)

    # p-major packed x: xP[st, p, c*ST+s] = x[s + st*ST, c*128 + p]
    x2d = np.asarray(hidden_states[0], np.float32)          # [S, HID]
    xP = np.ascontiguousarray(
        x2d.reshape(NST, ST, HC, 128).transpose(0, 3, 2, 1)
        .reshape(NST, 128, HC * ST)).astype(bf)
    pos = np.asarray(position_ids[0])
    sinT = np.ascontiguousarray(np.asarray(sin)[pos].T).astype(bf)
    cosT = np.ascontiguousarray(np.asarray(cos)[pos].T).astype(bf)
    ident = np.eye(128, dtype=bf)
    P = np.zeros((D, D), np.float32)
    for i in range(D // 2):
        P[i, i + D // 2] = -1.0
        P[i + D // 2, i] = 1.0
    pmat = np.ascontiguousarray(P.T).astype(bf)

    # fold ln weights into the column-sharded projections
    wqkv_full = (qkv_kernel * ln1_w[:, None]).astype(bf)    # [HID, OP]
    wgu_full = (gate_up_kernel * ln2_w[:, None]).astype(bf)  # [HID, 2*INTER]
    wo_full = o_kernel.astype(bf)                            # [HID, HID]
    wd_full = down_kernel.astype(bf)                         # [INTER, HID]

    in_maps = []
    for c in range(NCORES):
        qcols = wqkv_full[:, c * QH * D:(c + 1) * QH * D]
        kcols = wqkv_full[:, NH * D + c * D:NH * D + (c + 1) * D]
        vcols = wqkv_full[:, NH * D + NKV * D + c * D:
                          NH * D + NKV * D + (c + 1) * D]
        wqkv_c = np.concatenate([qcols, kcols, vcols], 1)      # [HID, OPC]
        # [m, p, hc*D]: tile m holds W[hc*128+p, m*D+o] at [p, hc*D+o]
        wqkv_t = np.ascontiguousarray(
            wqkv_c.reshape(HC, 128, QH + 2, D).transpose(2, 1, 0, 3)
            .reshape(QH + 2, 128, HC * D))
        wo_c = wo_full[c * DMC:(c + 1) * DMC, :]               # [384, HID]
        wo_t = np.ascontiguousarray(
            wo_c.reshape(3, 128, HC, 128).transpose(2, 1, 0, 3)
            .reshape(HC, 128, 3 * 128))
        gslice = wgu_full[:, c * DIC:(c + 1) * DIC]            # [HID, 1024]
        uslice = wgu_full[:, INTER + c * DIC:INTER + (c + 1) * DIC]
        wgu_gt = np.ascontiguousarray(
            gslice.reshape(HC, 128, DIC // 128, 128).transpose(2, 1, 0, 3)
            .reshape(DIC // 128, 128, HID))
        wgu_ut = np.ascontiguousarray(
            uslice.reshape(HC, 128, DIC // 128, 128).transpose(2, 1, 0, 3)
            .reshape(DIC // 128, 128, HID))
        wd_c = wd_full[c * DIC:(c + 1) * DIC, :]               # [1024, HID]
        wd_t = np.ascontiguousarray(
            wd_c.reshape(DIC // 128, 128, HC, 128).transpose(2, 1, 0, 3)
            .reshape(HC, 128, DIC))
        in_maps.append(dict(
            xP=xP, wqkv=wqkv_t, wo=wo_t, wgu_g=wgu_gt, wgu_u=wgu_ut, wd=wd_t,
            sinT=sinT, cosT=cosT, ident=ident, pmat=pmat,
            biasp=biasp,
        ))

    res = bass_utils.run_bass_kernel_spmd(nc, in_maps,
                                          core_ids=list(range(NCORES)))
    # assemble [HID, S]: cols 0:1024 from out_shard (rows 384c..384c+384),
    # cols 1024:2048 from the split-RS halves of tiles 2 and 3
    outT = np.empty((HID, S), np.float32)
    ra = RS_SPLIT_ROWS // NCORES                              # 192
    for c in range(NCORES):
        sh = np.asarray(res.results[c]["out_shard"], np.float32)
        outT[c * DMC:(c + 1) * DMC, 0:2 * ST] = sh
        for st in (2, 3):
            csl = slice(st * ST, (st + 1) * ST)
            a = np.asarray(res.results[c][f"o{st}a"], np.float32)
            b = np.asarray(res.results[c][f"o{st}b"], np.float32)
            outT[c * ra:(c + 1) * ra, csl] = a
            outT[RS_SPLIT_ROWS + c * ra:RS_SPLIT_ROWS + (c + 1) * ra,
                 csl] = b
    return np.ascontiguousarray(outT.T)[None].astype(np.float32)
